# revision 10
# baseline (speedup 1.0000x reference)
"""Two-layer GAT on 8 Trainium2 NeuronCores (Bass/Tile).

Strategy (dst-sharded, per the graph-partitioning hint):
  - Nodes are sharded contiguously across 8 cores (12500/core), and within
    each core locally re-ordered by in-degree so that each 128-row tile has
    near-uniform degree; tiles are padded to their max degree (2% overhead).
  - Each core projects its own nodes (x @ W1 on PE), computes per-node
    attention scalars, and writes a packed bf16 "gather table" row per node:
    [h (64) | a_src (8)].  An AllGather shares the table with all cores.
  - Edges live on the core that owns their destination.  For each dst tile
    the sources are gathered with one indirect DMA per chunk into a dense
    [128 dst, slots, feat] layout; segment softmax + weighted aggregation
    are dense vector ops (pad slots point at a dummy row with a_src=-1e4 so
    exp()==0 exactly).
  - Same again for layer 2 (41 classes, 1 head), then log_softmax.

kernel(**inputs) takes the full unsharded inputs and returns the full
[100000, 41] output; sharding/permutation are handled host-side here.
"""
import math
import os

import numpy as np

import concourse.bass as bass
import concourse.tile as tile
from concourse import bacc, mybir
from concourse.bass import IndirectOffsetOnAxis
from concourse.masks import make_identity

F32 = mybir.dt.float32
BF16 = mybir.dt.bfloat16
I32 = mybir.dt.int32

P = 128
NEG_SLOPE = 0.2
EPS = 1e-16
BIG_NEG = -1.0e4


def full_cfg():
    return dict(
        N=100000, E=1600000, F_IN=602, F_PAD=640,
        H1=8, C1=8, HC1=64, C2=41,
        NC=8, NPC_RAW=12500, T=98, CPR=8,
        MAXC=128,           # max padded gather columns per chunk
    )


def derive(cfg):
    cfg = dict(cfg)
    cfg["NPC"] = cfg["T"] * P
    cfg["NTOT"] = cfg["NC"] * cfg["NPC"]
    cfg["KC"] = cfg["F_PAD"] // P
    cfg["D1"] = cfg["HC1"] + cfg["H1"]      # 72 table-1 row width
    cfg["D2"] = cfg["C2"] + 1               # 42 table-2 row width
    return cfg


# ---------------------------------------------------------------- host prep
def preprocess(edge_index, cfg):
    c = derive(cfg)
    N, NC, NPC_RAW, T = c["N"], c["NC"], c["NPC_RAW"], c["T"]
    NPC = c["NPC"]
    CPR = c["CPR"]                      # cores per gather range
    NR = (NC + CPR - 1) // CPR          # number of ranges
    RROWS = CPR * NPC                   # table rows per range
    assert NR == 1 or RROWS <= 32767
    src = np.asarray(edge_index[0], dtype=np.int64)
    dst = np.asarray(edge_index[1], dtype=np.int64)
    owner = dst // NPC_RAW

    deg = np.bincount(dst, minlength=N)

    perm_old2new = np.empty(N, dtype=np.int64)
    new2old = np.empty((NC, NPC_RAW), dtype=np.int64)
    core_S = np.zeros((NC, T), dtype=np.int64)
    for cc in range(NC):
        lo = cc * NPC_RAW
        nodes = np.arange(lo, lo + NPC_RAW)
        order = np.argsort(-deg[lo:lo + NPC_RAW], kind="stable")
        sorted_nodes = nodes[order]
        new2old[cc] = sorted_nodes
        perm_old2new[sorted_nodes] = cc * NPC + np.arange(NPC_RAW)
        d_sorted = deg[lo:lo + NPC_RAW][order]
        d_pad = np.concatenate([d_sorted, np.zeros(NPC - NPC_RAW, np.int64)])
        core_S[cc] = d_pad.reshape(T, P).max(axis=1)

    # per-(core,node,range) counts -> per-tile max single-range count
    cnts = np.zeros((NC * NPC, NR), np.int64)
    src_new = perm_old2new[src]
    dst_new = perm_old2new[dst]
    rng_of_src = src_new // RROWS
    np.add.at(cnts, (dst_new, rng_of_src), 1)
    tile_maxb = cnts.reshape(NC, T, P, NR).max(axis=(2, 3))  # [NC, T]
    SB_tile = np.maximum(tile_maxb.max(axis=0), 1)           # [T]

    # group tiles into runs of equal total-S (as before) but the padded
    # profile is SB per range; make SB uniform within each legacy group
    S_tot = np.maximum(core_S.max(axis=0), 1)
    groups = []
    t = 0
    while t < T:
        gend = t
        while gend < T and S_tot[gend] == S_tot[t]:
            gend += 1
        groups.append((t, gend))
        t = gend
    SB = np.zeros(T, np.int64)
    for (a, b) in groups:
        SB[a:b] = SB_tile[a:b].max()
    S = SB * NR                                   # slots per tile
    CT = int(S.sum())
    col0 = np.concatenate([[0], np.cumsum(S)]).astype(np.int64)

    # per-core slot content: [NPC, NR, SBmax]
    SBmax = int(SB.max())
    idx_g = np.empty((NC, P, CT), dtype=np.int32)   # global ids (np model)
    chunks = make_chunks(SB, col0, cfg["MAXC"] // NR)
    # idx16 entry stream per core, in (chunk, range, tile, slot, p) order
    n16 = 128 * CT
    idx16 = (np.empty((NC, 16, n16 // 16), dtype=np.int16) if NR > 1
             else np.zeros((NC, 16, 1), dtype=np.int16))
    for cc in range(NC):
        pad_row = cc * NPC + NPC_RAW
        slot_tab = np.full((NPC, NR, SBmax), -1, dtype=np.int64)
        em = owner == cc
        es = src_new[em]
        eb = rng_of_src[em]
        ldst = dst_new[em] - cc * NPC
        order = np.lexsort((eb, ldst))
        ldst_s, es_s, eb_s = ldst[order], es[order], eb[order]
        key = ldst_s * NR + eb_s
        counts = np.bincount(key, minlength=NPC * NR)
        starts = np.concatenate([[0], np.cumsum(counts)])[:-1]
        slot = np.arange(len(key)) - starts[key]
        slot_tab[ldst_s, eb_s, slot] = es_s
        # global-id view (pad -> own pad row) for validation model
        st = slot_tab.reshape(T, P, NR, SBmax)
        ptr = 0
        stream = np.empty(n16 if NR > 1 else 0, np.int16)
        for (c0g, g0, n, sb) in chunks:
            if NR > 1:
                for b in range(NR):
                    blk = st[g0:g0 + n, :, b, :sb]          # [n, P, sb]
                    loc = np.where(blk >= 0, blk - b * RROWS, NPC_RAW)
                    ent = loc.transpose(0, 2, 1).reshape(-1)  # (t, s, p)
                    stream[ptr:ptr + ent.size] = ent.astype(np.int16)
                    ptr += ent.size
            for trel in range(n):
                t0 = g0 + trel
                full = st[t0, :, :, :]                   # [P, NR, SBmax]
                for b in range(NR):
                    blk = full[:, b, :sb]
                    gidx = np.where(blk >= 0, blk, pad_row)
                    idx_g[cc, :, col0[t0] + b * sb:
                          col0[t0] + (b + 1) * sb] = gidx
        if NR > 1:
            assert ptr == n16
            idx16[cc] = stream.reshape(n16 // 16, 16).T
    return dict(perm_old2new=perm_old2new, new2old=new2old,
                S=S, SB=SB, idx=idx_g, idx16=idx16, CT=CT, col0=col0,
                NR=NR, RROWS=RROWS, chunks=chunks)


def make_chunks(S, col0, MAXC):
    """[(col_start, first_tile, n_tiles, S)] — whole tiles, uniform S."""
    T = len(S)
    chunks = []
    t = 0
    while t < T:
        s = int(S[t])
        gend = t
        while gend < T and int(S[gend]) == s:
            gend += 1
        step = max(1, min(MAXC // s, 16))
        g0 = t
        while g0 < gend:
            n = min(step, gend - g0)
            chunks.append((int(col0[g0]), g0, n, s))
            g0 += n
        t = gend
    return chunks


# ---------------------------------------------------------------- program
def build_program(cfg, pre, n_cores=None, enable_asserts=False):
    c = derive(cfg)
    NC = n_cores if n_cores is not None else c["NC"]
    T, NPC, KC = c["T"], c["NPC"], c["KC"]
    NTOT = NC * NPC
    HC1, H1, C1, C2 = c["HC1"], c["H1"], c["C1"], c["C2"]
    CT = pre["CT"]
    NR, RROWS = pre["NR"], pre["RROWS"]
    chunks = pre["chunks"]
    DT = 128                                    # table row elems (bf16)
    HNPC = NPC // 2
    HT = T // 2
    assert T % 2 == 0
    dummy_p0 = c["NPC_RAW"] - (T - 1) * P      # dummy rows start in last tile
    assert 0 < dummy_p0 <= P  # pad row (= NPC_RAW) must be a dummy row

    nc = bacc.Bacc("TRN2", target_bir_lowering=False, debug=False,
                   enable_asserts=enable_asserts, num_devices=NC)

    xT = nc.dram_tensor("xT", [c["F_PAD"], NPC], BF16, kind="ExternalInput")
    W1 = nc.dram_tensor("W1", [c["F_PAD"], HC1], BF16, kind="ExternalInput")
    W2 = nc.dram_tensor("W2", [HC1, C2], BF16, kind="ExternalInput")
    att1s = nc.dram_tensor("att1s", [P, HC1], F32, kind="ExternalInput")
    att1d = nc.dram_tensor("att1d", [P, HC1], F32, kind="ExternalInput")
    att2s = nc.dram_tensor("att2s", [P, C2], F32, kind="ExternalInput")
    att2d = nc.dram_tensor("att2d", [P, C2], F32, kind="ExternalInput")
    b1 = nc.dram_tensor("b1", [P, HC1], F32, kind="ExternalInput")
    b2 = nc.dram_tensor("b2", [P, C2], F32, kind="ExternalInput")
    N16 = 128 * CT
    if NR > 1:
        idx_d = nc.dram_tensor("idx16", [16, N16 // 16], mybir.dt.int16,
                               kind="ExternalInput")
    else:
        idx_d = nc.dram_tensor("idx", [P, CT], I32, kind="ExternalInput")
    dneg_d = nc.dram_tensor("dneg", [P, 1], F32, kind="ExternalInput")
    out_d = nc.dram_tensor("out", [NPC, C2], F32, kind="ExternalOutput")

    rg = [list(range(NC))]
    AX = mybir.AxisListType.X
    OP = mybir.AluOpType
    ACT = mybir.ActivationFunctionType

    with tile.TileContext(nc) as tc:
        with (
            tc.tile_pool(name="dram", bufs=1, space="DRAM") as dramp,
            tc.tile_pool(name="state", bufs=1) as st,
        ):
            aspace = "Shared" if NC > 4 else "Local"
            tb1_loc = dramp.tile([NPC, DT], BF16)
            tb1_glob = dramp.tile([NTOT, DT], BF16, addr_space=aspace)
            tb2_loc = dramp.tile([NPC, DT], BF16)
            tb2_glob = dramp.tile([NTOT, DT], BF16, addr_space=aspace)
            adst_sb = st.tile([P, T * H1], F32)
            out1_sb = st.tile([P, T * HC1], F32)
            den_sb = st.tile([P, T * H1], F32)
            adst2_sb = st.tile([P, T], F32)
            out2_sb = st.tile([P, T * C2], F32)
            den2_sb = st.tile([P, T], F32)

            dneg_sb = st.tile([P, 1], F32)
            nc.sync.dma_start(out=dneg_sb[:], in_=dneg_d[:])
            if NR == 1:
                idx_sb = st.tile([P, CT], I32)
                nc.sync.dma_start(out=idx_sb[:], in_=idx_d[:])

            # ---------------- phase A: layer-1 projection -----------------
            with (
                tc.tile_pool(name="projA", bufs=1) as pA,
                tc.tile_pool(name="psA", bufs=4, space="PSUM") as psp,
            ):
                w1_sb = pA.tile([P, KC * HC1], BF16)
                nc.sync.dma_start(
                    out=w1_sb[:],
                    in_=W1[:].rearrange("(k p) c -> p k c", p=P))
                a1s_sb = pA.tile([P, HC1], F32)
                nc.sync.dma_start(out=a1s_sb[:], in_=att1s[:])
                a1d_sb = pA.tile([P, HC1], F32)
                nc.sync.dma_start(out=a1d_sb[:], in_=att1d[:])

                h_sb = pA.tile([P, T * HC1], F32)
                asrc_sb = pA.tile([P, T * H1], F32)

                for half in range(2):
                    xk = []
                    for k in range(KC):
                        xt = pA.tile([P, HNPC], BF16, tag=f"xk{k}")
                        nc.sync.dma_start(
                            out=xt[:],
                            in_=xT[k * P:(k + 1) * P,
                                   half * HNPC:(half + 1) * HNPC])
                        xk.append(xt)
                    for tl in range(HT):
                        t = half * HT + tl
                        ps = psp.tile([P, HC1], F32, tag="proj")
                        for k in range(KC):
                            nc.tensor.matmul(
                                ps[:],
                                lhsT=xk[k][:, tl * P:(tl + 1) * P],
                                rhs=w1_sb[:, k * HC1:(k + 1) * HC1],
                                start=(k == 0), stop=(k == KC - 1))
                        nc.vector.tensor_copy(
                            h_sb[:, t * HC1:(t + 1) * HC1], ps[:])

                # a_src / a_dst per node
                hv = h_sb[:].rearrange("p (t c) -> p t c", c=HC1)
                tmp = pA.tile([P, T * HC1], F32)
                tmpv = tmp[:].rearrange("p (t c) -> p t c", c=HC1)
                a1s_b = a1s_sb[:].rearrange("p c -> p () c") \
                    .to_broadcast([P, T, HC1])
                nc.vector.tensor_tensor(out=tmpv, in0=hv, in1=a1s_b,
                                        op=OP.mult)
                nc.vector.tensor_reduce(
                    out=asrc_sb[:].rearrange("p (t h) -> p t h", h=H1),
                    in_=tmp[:].rearrange("p (t h c) -> p t h c", h=H1, c=C1),
                    axis=AX, op=OP.add)
                a1d_b = a1d_sb[:].rearrange("p c -> p () c") \
                    .to_broadcast([P, T, HC1])
                nc.vector.tensor_tensor(out=tmpv, in0=hv, in1=a1d_b,
                                        op=OP.mult)
                nc.vector.tensor_reduce(
                    out=adst_sb[:].rearrange("p (t h) -> p t h", h=H1),
                    in_=tmp[:].rearrange("p (t h c) -> p t h c", h=H1, c=C1),
                    axis=AX, op=OP.add)
                # dummy rows: a_src += BIG_NEG so their exp() vanishes
                nc.vector.tensor_tensor(
                    out=asrc_sb[:, (T - 1) * H1:T * H1],
                    in0=asrc_sb[:, (T - 1) * H1:T * H1],
                    in1=dneg_sb[:].to_broadcast([P, H1]),
                    op=OP.add)

                # packed bf16 table row: [h | a_src]
                nc.gpsimd.dma_start(
                    out=tb1_loc[:, 0:HC1].rearrange("(t p) c -> p t c", p=P),
                    in_=hv)
                nc.gpsimd.dma_start(
                    out=tb1_loc[:, HC1:HC1 + H1]
                        .rearrange("(t p) c -> p t c", p=P),
                    in_=asrc_sb[:].rearrange("p (t h) -> p t h", h=H1))

            nc.gpsimd.collective_compute(
                "AllGather", OP.bypass, replica_groups=rg,
                ins=[tb1_loc[:].opt()], outs=[tb1_glob[:].opt()])

            # ---------------- phase B: layer-1 aggregation ----------------
            with tc.tile_pool(name="edge1", bufs=2) as pB:
                for (c0, g0, n, sb) in chunks:
                    CC = n * NR * sb
                    BL = n * sb                    # cols per range block
                    gb = pB.tile([P, CC * DT], BF16, tag="gb1")
                    gbv = gb[:].rearrange("p (c d) -> p c d", d=DT)
                    if NR == 1:
                        for col in range(CC):
                            nc.gpsimd.indirect_dma_start(
                                out=gbv[:, col, :], out_offset=None,
                                in_=tb1_glob[:],
                                in_offset=bass.IndirectOffsetOnAxis(
                                    ap=idx_sb[:, c0 + col:c0 + col + 1],
                                    axis=0))
                    else:
                        idxt = pB.tile([P, CC * 8], mybir.dt.int16, tag="ix1")
                        nc.sync.dma_start(
                            out=idxt[:],
                            in_=idx_d[:, c0 * 8:(c0 + CC) * 8]
                                .rearrange("q l -> () q l")
                                .to_broadcast([8, 16, CC * 8]))
                        for b in range(NR):
                            L = BL * P
                            nc.gpsimd.dma_gather(
                                gbv[:, b * BL:(b + 1) * BL, :],
                                tb1_glob[b * RROWS:(b + 1) * RROWS, :],
                                idxt[:, b * BL * 8:(b + 1) * BL * 8],
                                L, L, DT)
                    el = pB.tile([P, CC * H1], F32, tag="el")
                    elv = el[:].rearrange("p (c h) -> p c h", h=H1)
                    adst_b = adst_sb[:] \
                        .rearrange("p (t h) -> p t h", h=H1)[:, g0:g0 + n] \
                        .rearrange("p t h -> p t () h") \
                        .to_broadcast([P, n, sb, H1])
                    for b in range(NR):
                        nc.vector.tensor_tensor(
                            out=elv[:, b * BL:(b + 1) * BL, :]
                                .rearrange("p (t s) h -> p t s h", s=sb),
                            in0=gbv[:, b * BL:(b + 1) * BL, HC1:HC1 + H1]
                                .rearrange("p (t s) h -> p t s h", s=sb),
                            in1=adst_b, op=OP.add)
                    # w = exp(leaky_relu(el)) == max(exp(el), exp(0.2*el))
                    e1 = pB.tile([P, CC * H1], F32, tag="e1")
                    nc.scalar.activation(e1[:], el[:], ACT.Exp)
                    e2 = pB.tile([P, CC * H1], F32, tag="e2")
                    nc.scalar.activation(e2[:], el[:], ACT.Exp,
                                         scale=NEG_SLOPE)
                    w = pB.tile([P, CC * H1], F32, tag="el")
                    nc.vector.tensor_tensor(out=w[:], in0=e1[:], in1=e2[:],
                                            op=OP.max)
                    den4 = pB.tile([P, n * H1 * NR], F32, tag="d4")
                    d4v = den4[:].rearrange("p (t h b) -> p t h b", h=H1,
                                            b=NR)
                    for b in range(NR):
                        nc.vector.tensor_reduce(
                            out=d4v[:, :, :, b].rearrange("p t h -> p t h ()"),
                            in_=w[:].rearrange("p (c h) -> p c h", h=H1)
                                [:, b * BL:(b + 1) * BL, :]
                                .rearrange("p (t s) h -> p t h s", s=sb),
                            axis=AX, op=OP.add)
                    nc.vector.tensor_reduce(
                        out=den_sb[:, g0 * H1:(g0 + n) * H1]
                            .rearrange("p (t h) -> p t h", h=H1),
                        in_=d4v, axis=AX, op=OP.add)
                    mw = pB.tile([P, CC * HC1], BF16, tag="mw")
                    nc.vector.tensor_tensor(
                        out=mw[:].rearrange("p (c h e) -> p c h e",
                                            h=H1, e=C1),
                        in0=gbv[:, :, 0:HC1]
                            .rearrange("p c (h e) -> p c h e", e=C1),
                        in1=w[:].rearrange("p (c h) -> p c h ()", h=H1)
                            .to_broadcast([P, CC, H1, C1]),
                        op=OP.mult)
                    ou4 = pB.tile([P, n * HC1 * NR], F32, tag="o4")
                    o4v = ou4[:].rearrange("p (t e b) -> p t e b", e=HC1,
                                           b=NR)
                    for b in range(NR):
                        nc.vector.tensor_reduce(
                            out=o4v[:, :, :, b].rearrange("p t e -> p t e ()"),
                            in_=mw[:].rearrange("p (c e) -> p c e", e=HC1)
                                [:, b * BL:(b + 1) * BL, :]
                                .rearrange("p (t s) e -> p t e s", s=sb),
                            axis=AX, op=OP.add)
                    nc.vector.tensor_reduce(
                        out=out1_sb[:, g0 * HC1:(g0 + n) * HC1]
                            .rearrange("p (t e) -> p t e", e=HC1),
                        in_=o4v, axis=AX, op=OP.add)

            # ---------------- phase C: finalize L1, project L2 ------------
            with (
                tc.tile_pool(name="mid", bufs=1) as pC,
                tc.tile_pool(name="psC", bufs=3, space="PSUM") as psp,
            ):
                idn = pC.tile([P, P], BF16)
                make_identity(nc, idn[:])
                w2_sb = pC.tile([HC1, C2], BF16)
                nc.sync.dma_start(out=w2_sb[:], in_=W2[:])
                a2s_sb = pC.tile([P, C2], F32)
                nc.sync.dma_start(out=a2s_sb[:], in_=att2s[:])
                a2d_sb = pC.tile([P, C2], F32)
                nc.sync.dma_start(out=a2d_sb[:], in_=att2d[:])
                b1_sb = pC.tile([P, HC1], F32)
                nc.sync.dma_start(out=b1_sb[:], in_=b1[:])

                dee = pC.tile([P, T * H1], F32)
                nc.vector.tensor_scalar_add(dee[:], den_sb[:], EPS)
                rec = pC.tile([P, T * H1], F32)
                nc.vector.reciprocal(rec[:], dee[:])
                on = out1_sb  # normalize in place
                nc.vector.tensor_tensor(
                    out=on[:].rearrange("p (t h e) -> p t h e", h=H1, e=C1),
                    in0=out1_sb[:].rearrange("p (t h e) -> p t h e",
                                             h=H1, e=C1),
                    in1=rec[:].rearrange("p (t h) -> p t h ()", h=H1)
                        .to_broadcast([P, T, H1, C1]),
                    op=OP.mult)
                nc.vector.tensor_tensor(
                    out=on[:].rearrange("p (t e) -> p t e", e=HC1),
                    in0=on[:].rearrange("p (t e) -> p t e", e=HC1),
                    in1=b1_sb[:].rearrange("p e -> p () e")
                        .to_broadcast([P, T, HC1]),
                    op=OP.add)
                # ELU -> bf16
                mn = pC.tile([P, T * HC1], F32, tag="celu")
                nc.vector.tensor_scalar_min(mn[:], on[:], 0.0)
                ex = pC.tile([P, T * HC1], F32)
                nc.scalar.activation(ex[:], mn[:], ACT.Exp)
                mx1 = pC.tile([P, T * HC1], F32, tag="celu")
                nc.vector.tensor_scalar(mx1[:], on[:], 0.0, -1.0,
                                        OP.max, OP.add)
                h2in = pC.tile([P, T * HC1], BF16)
                nc.vector.tensor_tensor(out=h2in[:], in0=mx1[:], in1=ex[:],
                                        op=OP.add)

                # transpose tiles -> [HC1, NPC]
                h2inT = pC.tile([HC1, NPC], BF16)
                for t in range(T):
                    pst = psp.tile([HC1, P], BF16, tag="tr")
                    nc.tensor.transpose(
                        pst[:], h2in[:, t * HC1:(t + 1) * HC1], idn[:])
                    nc.vector.tensor_copy(h2inT[:, t * P:(t + 1) * P], pst[:])

                h2_sb = pC.tile([P, T * C2], F32)
                for t in range(T):
                    ps2 = psp.tile([P, C2], F32, tag="proj2")
                    nc.tensor.matmul(ps2[:],
                                     lhsT=h2inT[:, t * P:(t + 1) * P],
                                     rhs=w2_sb[:], start=True, stop=True)
                    nc.vector.tensor_copy(h2_sb[:, t * C2:(t + 1) * C2],
                                          ps2[:])

                h2v = h2_sb[:].rearrange("p (t c) -> p t c", c=C2)
                tmp2 = pC.tile([P, T * C2], F32)
                tmp2v = tmp2[:].rearrange("p (t c) -> p t c", c=C2)
                a2s_b = a2s_sb[:].rearrange("p c -> p () c") \
                    .to_broadcast([P, T, C2])
                nc.vector.tensor_tensor(out=tmp2v, in0=h2v, in1=a2s_b,
                                        op=OP.mult)
                asrc2_sb = pC.tile([P, T], F32)
                nc.vector.tensor_reduce(
                    out=asrc2_sb[:].rearrange("p t -> p t ()"),
                    in_=tmp2v, axis=AX, op=OP.add)
                a2d_b = a2d_sb[:].rearrange("p c -> p () c") \
                    .to_broadcast([P, T, C2])
                nc.vector.tensor_tensor(out=tmp2v, in0=h2v, in1=a2d_b,
                                        op=OP.mult)
                nc.vector.tensor_reduce(
                    out=adst2_sb[:].rearrange("p t -> p t ()"),
                    in_=tmp2v, axis=AX, op=OP.add)
                nc.vector.tensor_tensor(
                    out=asrc2_sb[:, T - 1:T], in0=asrc2_sb[:, T - 1:T],
                    in1=dneg_sb[:], op=OP.add)

                nc.gpsimd.dma_start(
                    out=tb2_loc[:, 0:C2].rearrange("(t p) c -> p t c", p=P),
                    in_=h2v)
                nc.gpsimd.dma_start(
                    out=tb2_loc[:, C2:C2 + 1].rearrange("(t p) c -> p t c", p=P),
                    in_=asrc2_sb[:].rearrange("p t -> p t ()"))

            nc.gpsimd.collective_compute(
                "AllGather", OP.bypass, replica_groups=rg,
                ins=[tb2_loc[:].opt()], outs=[tb2_glob[:].opt()])

            # ---------------- phase D: layer-2 aggregation ----------------
            with tc.tile_pool(name="edge2", bufs=2) as pD:
                for (c0, g0, n, sb) in chunks:
                    CC = n * NR * sb
                    BL = n * sb
                    gb = pD.tile([P, CC * DT], BF16, tag="gb2")
                    gbv = gb[:].rearrange("p (c d) -> p c d", d=DT)
                    if NR == 1:
                        for col in range(CC):
                            nc.gpsimd.indirect_dma_start(
                                out=gbv[:, col, :], out_offset=None,
                                in_=tb2_glob[:],
                                in_offset=bass.IndirectOffsetOnAxis(
                                    ap=idx_sb[:, c0 + col:c0 + col + 1],
                                    axis=0))
                    else:
                        idxt = pD.tile([P, CC * 8], mybir.dt.int16, tag="ix2")
                        nc.sync.dma_start(
                            out=idxt[:],
                            in_=idx_d[:, c0 * 8:(c0 + CC) * 8]
                                .rearrange("q l -> () q l")
                                .to_broadcast([8, 16, CC * 8]))
                        for b in range(NR):
                            L = BL * P
                            nc.gpsimd.dma_gather(
                                gbv[:, b * BL:(b + 1) * BL, :],
                                tb2_glob[b * RROWS:(b + 1) * RROWS, :],
                                idxt[:, b * BL * 8:(b + 1) * BL * 8],
                                L, L, DT)
                    el = pD.tile([P, CC], F32, tag="el2")
                    elv = el[:].rearrange("p (t s) -> p t s", s=sb)
                    adst_b = adst2_sb[:, g0:g0 + n] \
                        .rearrange("p t -> p t ()").to_broadcast([P, n, sb])
                    for b in range(NR):
                        nc.vector.tensor_tensor(
                            out=el[:, b * BL:(b + 1) * BL]
                                .rearrange("p (t s) -> p t s", s=sb),
                            in0=gbv[:, b * BL:(b + 1) * BL, C2:C2 + 1]
                                .rearrange("p (t s) h -> p t (s h)", s=sb),
                            in1=adst_b, op=OP.add)
                    e1 = pD.tile([P, CC], F32, tag="e12")
                    nc.scalar.activation(e1[:], el[:], ACT.Exp)
                    e2 = pD.tile([P, CC], F32, tag="e22")
                    nc.scalar.activation(e2[:], el[:], ACT.Exp,
                                         scale=NEG_SLOPE)
                    w = pD.tile([P, CC], F32, tag="el2")
                    nc.vector.tensor_tensor(out=w[:], in0=e1[:], in1=e2[:],
                                            op=OP.max)
                    den4 = pD.tile([P, n * NR], F32, tag="d42")
                    d4v = den4[:].rearrange("p (t b) -> p t b", b=NR)
                    for b in range(NR):
                        nc.vector.tensor_reduce(
                            out=d4v[:, :, b].rearrange("p t -> p t ()"),
                            in_=w[:, b * BL:(b + 1) * BL]
                                .rearrange("p (t s) -> p t s", s=sb),
                            axis=AX, op=OP.add)
                    nc.vector.tensor_reduce(
                        out=den2_sb[:, g0:g0 + n].rearrange("p t -> p t ()"),
                        in_=d4v, axis=AX, op=OP.add)
                    mw = pD.tile([P, CC * C2], BF16, tag="mw2")
                    nc.vector.tensor_tensor(
                        out=mw[:].rearrange("p (c e) -> p c e", e=C2),
                        in0=gbv[:, :, 0:C2],
                        in1=w[:].rearrange("p c -> p c ()")
                            .to_broadcast([P, CC, C2]),
                        op=OP.mult)
                    ou4 = pD.tile([P, n * C2 * NR], F32, tag="o42")
                    o4v = ou4[:].rearrange("p (t e b) -> p t e b", e=C2, b=NR)
                    for b in range(NR):
                        nc.vector.tensor_reduce(
                            out=o4v[:, :, :, b].rearrange("p t e -> p t e ()"),
                            in_=mw[:].rearrange("p (c e) -> p c e", e=C2)
                                [:, b * BL:(b + 1) * BL, :]
                                .rearrange("p (t s) e -> p t e s", s=sb),
                            axis=AX, op=OP.add)
                    nc.vector.tensor_reduce(
                        out=out2_sb[:, g0 * C2:(g0 + n) * C2]
                            .rearrange("p (t e) -> p t e", e=C2),
                        in_=o4v, axis=AX, op=OP.add)

            # ---------------- phase E: normalize + log_softmax ------------
            with tc.tile_pool(name="fin", bufs=1) as pE:
                b2_sb = pE.tile([P, C2], F32)
                nc.sync.dma_start(out=b2_sb[:], in_=b2[:])
                dee2 = pE.tile([P, T], F32)
                nc.vector.tensor_scalar_add(dee2[:], den2_sb[:], EPS)
                rec2 = pE.tile([P, T], F32)
                nc.vector.reciprocal(rec2[:], dee2[:])
                o2 = pE.tile([P, T * C2], F32)
                o2v = o2[:].rearrange("p (t c) -> p t c", c=C2)
                nc.vector.tensor_tensor(
                    out=o2v,
                    in0=out2_sb[:].rearrange("p (t c) -> p t c", c=C2),
                    in1=rec2[:].rearrange("p t -> p t ()")
                        .to_broadcast([P, T, C2]),
                    op=OP.mult)
                nc.vector.tensor_tensor(
                    out=o2v, in0=o2v,
                    in1=b2_sb[:].rearrange("p c -> p () c")
                        .to_broadcast([P, T, C2]),
                    op=OP.add)
                mxt = pE.tile([P, T], F32)
                nc.vector.tensor_reduce(
                    out=mxt[:].rearrange("p t -> p t ()"),
                    in_=o2v, axis=AX, op=OP.max)
                nc.vector.tensor_tensor(
                    out=o2v, in0=o2v,
                    in1=mxt[:].rearrange("p t -> p t ()")
                        .to_broadcast([P, T, C2]),
                    op=OP.subtract)
                exs = pE.tile([P, T * C2], F32)
                nc.scalar.activation(exs[:], o2[:], ACT.Exp)
                ssum = pE.tile([P, T], F32)
                nc.vector.tensor_reduce(
                    out=ssum[:].rearrange("p t -> p t ()"),
                    in_=exs[:].rearrange("p (t c) -> p t c", c=C2),
                    axis=AX, op=OP.add)
                lns = pE.tile([P, T], F32)
                nc.scalar.activation(lns[:], ssum[:], ACT.Ln)
                fin = pE.tile([P, T * C2], F32)
                nc.vector.tensor_tensor(
                    out=fin[:].rearrange("p (t c) -> p t c", c=C2),
                    in0=o2v,
                    in1=lns[:].rearrange("p t -> p t ()")
                        .to_broadcast([P, T, C2]),
                    op=OP.subtract)
                nc.sync.dma_start(
                    out=out_d[:].rearrange("(t p) c -> p t c", p=P),
                    in_=fin[:].rearrange("p (t c) -> p t c", c=C2))

    nc.compile()
    return nc


# ---------------------------------------------------------------- host glue
def pack_inputs(inputs, pre, cfg, n_cores=None):
    import ml_dtypes
    c = derive(cfg)
    NC = n_cores if n_cores is not None else c["NC"]
    NPC, NPC_RAW = c["NPC"], c["NPC_RAW"]
    x = np.asarray(inputs["x"], np.float32)
    W1 = np.asarray(inputs["W1"], np.float32)
    W2 = np.asarray(inputs["W2"], np.float32)
    a1s = np.asarray(inputs["att_src1"], np.float32).reshape(-1)
    a1d = np.asarray(inputs["att_dst1"], np.float32).reshape(-1)
    a2s = np.asarray(inputs["att_src2"], np.float32).reshape(-1)
    a2d = np.asarray(inputs["att_dst2"], np.float32).reshape(-1)
    b1 = np.asarray(inputs["b1"], np.float32).reshape(-1)
    b2 = np.asarray(inputs["b2"], np.float32).reshape(-1)

    W1p = np.zeros((c["F_PAD"], c["HC1"]), np.float32)
    W1p[:c["F_IN"]] = W1
    dummy_p0 = c["NPC_RAW"] - (c["T"] - 1) * P
    dneg = np.zeros((P, 1), np.float32)
    dneg[dummy_p0:] = BIG_NEG
    maps = []
    for cc in range(NC):
        xg = np.zeros((NPC, c["F_IN"]), np.float32)
        xg[:NPC_RAW] = x[pre["new2old"][cc]]
        xTp = np.zeros((c["F_PAD"], NPC), ml_dtypes.bfloat16)
        xTp[:c["F_IN"]] = xg.T
        maps.append({
            "xT": xTp,
            "W1": W1p.astype(ml_dtypes.bfloat16),
            "W2": W2.astype(ml_dtypes.bfloat16),
            "att1s": np.tile(a1s, (P, 1)).astype(np.float32),
            "att1d": np.tile(a1d, (P, 1)).astype(np.float32),
            "att2s": np.tile(a2s, (P, 1)).astype(np.float32),
            "att2d": np.tile(a2d, (P, 1)).astype(np.float32),
            "b1": np.tile(b1, (P, 1)).astype(np.float32),
            "b2": np.tile(b2, (P, 1)).astype(np.float32),
            **({"idx16": pre["idx16"][cc]} if pre["NR"] > 1
               else {"idx": pre["idx"][cc]}),
            "dneg": dneg,
        })
    return maps


def unpack_output(results, pre, cfg, n_cores=None):
    c = derive(cfg)
    NC = n_cores if n_cores is not None else c["NC"]
    out = np.empty((c["N"], c["C2"]), np.float32)
    for cc in range(NC):
        out[pre["new2old"][cc]] = results[cc]["out"][:c["NPC_RAW"]]
    return out


_CACHE = {}


def kernel(**inputs) -> np.ndarray:
    from concourse.bass_utils import run_bass_kernel_spmd
    cfg = full_cfg()
    edge_key = id(inputs["edge_index"])
    pre = preprocess(inputs["edge_index"], cfg)
    key = (tuple(pre["S"].tolist()),)
    if key not in _CACHE:
        _CACHE[key] = build_program(cfg, pre)
    nc = _CACHE[key]
    in_maps = pack_inputs(inputs, pre, cfg)
    res = run_bass_kernel_spmd(nc, in_maps, core_ids=list(range(cfg["NC"])))
    return unpack_output(res.results, pre, cfg)


if __name__ == "__main__":
    # smoke-build at full scale (no execution)
    cfg = full_cfg()
    rng = np.random.default_rng(0)
    ei = rng.integers(0, cfg["N"], size=(2, cfg["E"]), dtype=np.int64)
    pre = preprocess(ei, cfg)
    print("CT", pre["CT"], "chunks", len(pre["chunks"]),
          "pad_x", 128 * pre["CT"] * cfg["NC"] / cfg["E"])
    nc = build_program(cfg, pre)
    print("build OK")
